# revision 15
# baseline (speedup 1.0000x reference)
"""Trainium2 Bass kernel for the OpenNMT-style decoder (2-layer LSTM + Luong attention).

Strategy (8 NeuronCores, one chip):
  - Scan phase: tensor-parallel over the 4H gate dimension (512 gate rows per core,
    grouped as [i|f|o|g] 128-row h-slices). Recurrent matmuls run with the hidden
    state as the stationary operand and the (pre-transposed) weights as the moving
    operand, so each round streams N=512 bf16 columns per K-tile. Round r computes
    gates0(r) -> h0(r) and gates1(r-1) -> h1(r-1); both depend only on the previous
    round's gathered [h0|h1], so a single 32KB AllGather per round suffices.
  - x-projection (W_ih0 @ emb(x) + biases) is computed on-device with a small
    lookahead so it fills PE gaps during the scan.
  - Attention is computed after the scan, sharded over batch (8 per core), using the
    per-core history of gathered h1 slices.
  - Embedding gather and weight layout/bf16 conversion happen on the host as part of
    input distribution; outputs are reassembled on the host.
"""

import numpy as np
import ml_dtypes

import concourse.bass as bass
import concourse.mybir as mybir
import concourse.tile as tile
from concourse import bacc
from concourse.bass import ds, ts
from concourse.bass_utils import run_bass_kernel_spmd

F32 = mybir.dt.float32
BF16 = mybir.dt.bfloat16

T, B, S = 64, 64, 64
H, E, V, L = 1024, 1024, 50000, 2
NC = 8
KT = H // 128            # 8 K-tiles per hidden vector
GS = 4 * H // NC         # 512 gate rows per core
BPC = B // NC            # 8 batch elements per core for attention
LA = 4                   # x-projection lookahead (rounds)
COLTILE = True           # concurrent column-tiled streams on the PE array

RG = [list(range(NC))]

_cached = {}


def _build_nc():
    nc = bacc.Bacc(None, target_bir_lowering=False, num_devices=NC)

    dram_in = {}

    def din(name, shape, dtype):
        dram_in[name] = nc.dram_tensor(name, shape, dtype, kind="ExternalInput")
        return dram_in[name]

    def dout(name, shape, dtype):
        return nc.dram_tensor(name, shape, dtype, kind="ExternalOutput")

    # per-core weight slices (pre-transposed on host)
    wA_d = din("wA", [H, GS], BF16)            # W_hh0[rows_k].T
    wB_d = din("wB", [2 * H, GS], BF16)        # [W_ih1 | W_hh1][rows_k].T
    wX_d = din("wX", [H, GS], BF16)            # W_ih0[rows_k].T
    b0_d = din("b0", [1, GS], BF16)            # (b_ih0+b_hh0)[rows_k]
    b1_d = din("b1", [1, GS], BF16)
    ones_d = din("ones1", [1, B], BF16)
    i128_d = din("i128", [128, 128], BF16)
    xT_d = din("xT", [T * H, B], BF16)         # per-step x(t).T, row-major [t, h, b]
    hinit_d = din("hinitg", [NC * 128, 128], BF16)   # gathered-format initial h0(-1)
    h1init_d = din("h1init", [B, 128], BF16)   # per-core h1(-1) slice, [b, h]
    cinit_d = din("cinit", [128, 128], F32)    # rows 0:64 c0 slice [b,h], 64:128 c1
    ctxT_d = din("ctxT", [128, KT * BPC * S], BF16)   # [p, kk, b, s]
    ctxS_d = din("ctxS", [S, BPC * KT * 128], BF16)   # [s, b, kk, m]
    winT_d = din("winT", [128, KT * KT * 128], BF16)  # [p, kk, mt, m]
    woutT_d = din("woutT", [128, 2 * KT * KT * 128], BF16)  # [p, kk(16), mt, m]

    out_d = dout("out_o", [KT * 128, BPC * T], F32)   # [mt*128+p, b*T+t]
    hf_d = dout("hf_o", [128, 128], BF16)      # 0:64 h0(63) [b,h], 64:128 h1(63)
    cf_d = dout("cf_o", [128, 128], F32)
    attn_d = dout("attn_o", [S, BPC], F32)

    tpA = (0, 0) if COLTILE else None
    tpB = (0, 64) if COLTILE else None

    with tile.TileContext(nc) as tc, tc.tile_pool(name="persist", bufs=1) as persist:
        with (
            tc.tile_pool(name="wpool", bufs=1) as wpool,
            tc.tile_pool(name="state", bufs=1) as state,
            tc.tile_pool(name="xstat", bufs=3) as xstat_pool,
            tc.tile_pool(name="xps", bufs=LA + 2) as xps_pool,
            tc.tile_pool(name="hp", bufs=2) as hp_pool,
            tc.tile_pool(name="send", bufs=2) as send_pool,
            tc.tile_pool(name="gath", bufs=3) as gath_pool,
            tc.tile_pool(name="cell", bufs=3) as cell_pool,
            tc.tile_pool(name="psg", bufs=2, space="PSUM") as psg_pool,
            tc.tile_pool(name="psx", bufs=2, space="PSUM") as psx_pool,
            tc.tile_pool(name="pst", bufs=2, space="PSUM") as pst_pool,
            tc.tile_pool(name="dram", bufs=2, space="DRAM") as dram_pool,
        ):
            # ---------------- weight / constant preload ----------------
            wA = wpool.tile([128, KT, GS], BF16)
            nc.sync.dma_start(out=wA[:], in_=wA_d.ap().rearrange("(k p) n -> p k n", p=128))
            wB = wpool.tile([128, 2 * KT, GS], BF16)
            nc.sync.dma_start(out=wB[:], in_=wB_d.ap().rearrange("(k p) n -> p k n", p=128))
            wX = wpool.tile([128, KT, GS], BF16)
            nc.sync.dma_start(out=wX[:], in_=wX_d.ap().rearrange("(k p) n -> p k n", p=128))
            b0 = wpool.tile([1, GS], BF16)
            nc.sync.dma_start(out=b0[:], in_=b0_d.ap())
            b1 = wpool.tile([1, GS], BF16)
            nc.sync.dma_start(out=b1[:], in_=b1_d.ap())
            ones1 = wpool.tile([1, B], BF16)
            nc.sync.dma_start(out=ones1[:], in_=ones_d.ap())
            i128 = persist.tile([128, 128], BF16)
            nc.sync.dma_start(out=i128[:], in_=i128_d.ap())
            h1init = wpool.tile([B, 128], BF16)
            nc.sync.dma_start(out=h1init[:], in_=h1init_d.ap())

            # persistent state
            c_sb = state.tile([128, 128], F32)            # 0:64 c0 [b,h], 64:128 c1
            nc.sync.dma_start(out=c_sb[:], in_=cinit_d.ap())
            hist = persist.tile([128, T, KT, BPC], BF16)  # h1 history [p, t, kk, b]

            # initial gathered h (round -1 exchange result)
            g_prev = gath_pool.tile([128, NC, 128], BF16, tag="gath")
            nc.sync.dma_start(
                out=g_prev[:], in_=hinit_d.ap().rearrange("(k p) f -> p k f", p=128)
            )

            # dynamic offset for the per-core hist slice of the gathered buffer
            pid = nc.vector.partition_id()
            hist_off = nc.snap(pid * BPC + 64)

            xp_tiles = {}

            def xp_block(q):
                # compute xp(q) = W_ih0 @ x(q) + b0, store bf16 in SBUF
                xs = xstat_pool.tile([128, KT, B], BF16, tag="xstat")
                nc.sync.dma_start(
                    out=xs[:],
                    in_=xT_d.ap()[ts(q, H), :].rearrange("(k p) b -> p k b", p=128),
                )
                px = psx_pool.tile([B, GS], F32, tag="psx")
                for kk in range(KT):
                    nc.tensor.matmul(
                        px[:], lhsT=xs[:, kk, :], rhs=wX[:, kk, :],
                        start=(kk == 0), stop=False, tile_position=tpA,
                    )
                nc.tensor.matmul(
                    px[:], lhsT=ones1[:], rhs=b0[:],
                    start=False, stop=True, tile_position=tpA,
                )
                xt = xps_pool.tile([B, GS], BF16, tag="xps")
                nc.vector.tensor_copy(xt[:], px[:])
                xp_tiles[q] = xt

            def cell(gates_ap, c_ap, h_ap, lo):
                # gates_ap: [64, 512] fp32 PSUM in [i|f|o|g] order, base partition lo
                # c_ap: [64, 128] fp32 SBUF (in/out) @lo, h_ap: [64, 128] bf16 out @lo
                sl = slice(lo, lo + 64)
                sig_t = cell_pool.tile([128, 3 * 128], F32, tag="sig")
                sig = sig_t[sl, :]
                nc.scalar.activation(sig, gates_ap[:, 0:384],
                                     mybir.ActivationFunctionType.Sigmoid)
                tg_t = cell_pool.tile([128, 128], F32, tag="tg")
                tg = tg_t[sl, :]
                nc.scalar.activation(tg, gates_ap[:, 384:512],
                                     mybir.ActivationFunctionType.Tanh)
                t1_t = cell_pool.tile([128, 128], F32, tag="t1")
                t1 = t1_t[sl, :]
                nc.vector.tensor_tensor(t1, sig[:, 0:128], tg,
                                        op=mybir.AluOpType.mult)
                t2_t = cell_pool.tile([128, 128], F32, tag="t2")
                t2 = t2_t[sl, :]
                nc.vector.tensor_tensor(t2, sig[:, 128:256], c_ap,
                                        op=mybir.AluOpType.mult)
                nc.vector.tensor_tensor(c_ap, t1, t2, op=mybir.AluOpType.add)
                tc_t = cell_pool.tile([128, 128], F32, tag="tc")
                tc_ = tc_t[sl, :]
                nc.scalar.activation(tc_, c_ap,
                                     mybir.ActivationFunctionType.Tanh)
                nc.vector.tensor_tensor(h_ap, sig[:, 256:384], tc_,
                                        op=mybir.AluOpType.mult)

            # xp prologue
            for q in range(min(LA, T)):
                xp_block(q)

            # ---------------- scan rounds ----------------
            for r in range(T + 1):
                if r + LA <= T - 1:
                    xp_block(r + LA)

                pg = psg_pool.tile([128, GS], F32, tag="psg")
                if r <= T - 1:
                    # stream A: gates0(r) = W_hh0 @ h0(r-1) + xp(r)
                    for kk in range(KT):
                        nc.tensor.matmul(
                            pg[0:64, :], lhsT=g_prev[:, kk, 0:64], rhs=wA[:, kk, :],
                            start=(kk == 0), stop=False, tile_position=tpA,
                            skip_group_check=True,
                        )
                    nc.tensor.matmul(
                        pg[0:64, :], lhsT=i128[0:64, 0:64], rhs=xp_tiles[r][:],
                        start=False, stop=True, tile_position=tpA,
                        skip_group_check=True,
                    )
                if r >= 1:
                    # stream B: gates1(r-1) = W_ih1 @ h0(r-1) + W_hh1 @ h1(r-2) + b1
                    for kk in range(KT):
                        nc.tensor.matmul(
                            pg[64:128, :], lhsT=g_prev[:, kk, 0:64], rhs=wB[:, kk, :],
                            start=(kk == 0), stop=False, tile_position=tpB,
                            skip_group_check=True,
                        )
                    for kk in range(KT):
                        nc.tensor.matmul(
                            pg[64:128, :], lhsT=g_prev[:, kk, 64:128],
                            rhs=wB[:, KT + kk, :],
                            start=False, stop=False, tile_position=tpB,
                            skip_group_check=True,
                        )
                    nc.tensor.matmul(
                        pg[64:128, :], lhsT=ones1[:], rhs=b1[:],
                        start=False, stop=True, tile_position=tpB,
                        skip_group_check=True,
                    )

                hp = hp_pool.tile([128, 128], BF16, tag="hp")
                if r == 0:
                    nc.vector.tensor_copy(hp[64:128, :], h1init[:])
                if r <= T - 1:
                    cell(pg[0:64, :], c_sb[0:64, :], hp[0:64, :], 0)
                if r >= 1:
                    cell(pg[64:128, :], c_sb[64:128, :], hp[64:128, :], 64)
                if r == T:
                    nc.vector.tensor_copy(hp[0:64, :], h1init[:])  # filler

                if r == T - 1:
                    nc.sync.dma_start(out=hf_d.ap()[0:64, :], in_=hp[0:64, :])
                if r == T:
                    nc.sync.dma_start(out=hf_d.ap()[64:128, :], in_=hp[64:128, :])

                ptr = pst_pool.tile([128, 128], BF16, tag="pst")
                nc.tensor.transpose(ptr[:], hp[:], i128[:])
                snd = send_pool.tile([128, 128], BF16, tag="send")
                nc.vector.tensor_copy(snd[:], ptr[:])

                dsend = dram_pool.tile([128, 128], BF16, tag="dsend")
                dgath = dram_pool.tile([NC * 128, 128], BF16, tag="dgath")
                nc.sync.dma_start(out=dsend[:], in_=snd[:])
                nc.gpsimd.collective_compute(
                    "AllGather", mybir.AluOpType.bypass, replica_groups=RG,
                    ins=[dsend[:].opt()], outs=[dgath[:].opt()],
                )
                g_new = gath_pool.tile([128, NC, 128], BF16, tag="gath")
                nc.sync.dma_start(
                    out=g_new[:], in_=dgath[:].rearrange("(k p) f -> p k f", p=128)
                )
                if r >= 1:
                    # h1(r-1) slices for my batch elements -> history
                    nc.vector.tensor_copy(
                        hist[:, r - 1, :, :], g_new[:, :, ds(hist_off, BPC)]
                    )
                g_prev = g_new

            nc.sync.dma_start(out=cf_d.ap(), in_=c_sb[:])

        # ---------------- attention phase ----------------
        with (
            tc.tile_pool(name="aw", bufs=1) as aw,
            tc.tile_pool(name="ascr", bufs=2) as ascr,
        ):
            ctxT = aw.tile([128, KT, BPC, S], BF16)
            nc.sync.dma_start(out=ctxT[:], in_=ctxT_d.ap().rearrange(
                "p (k b s) -> p k b s", k=KT, b=BPC))
            ctxS = aw.tile([S, BPC, KT, 128], BF16)
            nc.sync.dma_start(out=ctxS[:], in_=ctxS_d.ap().rearrange(
                "s (b k m) -> s b k m", b=BPC, k=KT))
            winT = aw.tile([128, KT, KT, 128], BF16)
            nc.sync.dma_start(out=winT[:], in_=winT_d.ap().rearrange(
                "p (k t m) -> p k t m", k=KT, t=KT))
            woutT = aw.tile([128, 2 * KT, KT, 128], BF16)
            nc.sync.dma_start(out=woutT[:], in_=woutT_d.ap().rearrange(
                "p (k t m) -> p k t m", k=2 * KT, t=KT))
            i64b = aw.tile([64, 64], BF16)
            nc.vector.tensor_copy(i64b[:], i128[0:64, 0:64])

            # ---- q projection: qT[p, b, kk, t] = (h1 @ w_in.T).T ----
            qT = aw.tile([128, BPC, KT, T], BF16)
            with tc.tile_pool(name="psqp", bufs=2, space="PSUM") as psq_pool:
                for b0 in range(0, BPC, 2):
                    psq0 = psq_pool.tile([128, KT * T], F32, tag="psq0")
                    psq1 = psq_pool.tile([128, KT * T], F32, tag="psq1")
                    pair = [psq0, psq1]
                    for mt in range(KT):
                        for kk in range(KT):
                            for j in range(2):
                                nc.tensor.matmul(
                                    pair[j][:, ts(mt, T)],
                                    lhsT=winT[:, kk, mt, :],
                                    rhs=hist[:, :, kk, b0 + j],
                                    start=(kk == 0), stop=(kk == KT - 1),
                                    skip_group_check=True,
                                )
                    for j in range(2):
                        nc.vector.tensor_copy(
                            qT[:, b0 + j, :, :],
                            pair[j][:].rearrange("p (k t) -> p k t", k=KT),
                        )

            # ---- scores + softmax ----
            expv = ascr.tile([64, BPC * S], BF16, tag="expv")
            with tc.tile_pool(name="pssp", bufs=1, space="PSUM") as pss_pool:
                pssc = pss_pool.tile([64, BPC * S], F32, tag="pssc")
                for b in range(BPC):
                    for kk in range(KT):
                        nc.tensor.matmul(
                            pssc[:, ts(b, S)], lhsT=qT[:, b, kk, :],
                            rhs=ctxT[:, kk, b, :],
                            start=(kk == 0), stop=(kk == KT - 1),
                            skip_group_check=True,
                        )
                nc.scalar.activation(expv[:], pssc[:],
                                     mybir.ActivationFunctionType.Exp)
            sums = ascr.tile([64, BPC], F32, tag="sums")
            nc.vector.tensor_reduce(
                sums[:], expv[:].rearrange("p (b s) -> p b s", b=BPC),
                axis=mybir.AxisListType.X, op=mybir.AluOpType.add,
            )
            rec = ascr.tile([64, BPC], F32, tag="rec")
            nc.vector.reciprocal(rec[:], sums[:])

            # ---- normalized A^T and weighted context ----
            wcT = aw.tile([128, BPC, KT, T], BF16)
            attn_sb = ascr.tile([S, BPC], F32, tag="attnsb")
            with tc.tile_pool(name="psap", bufs=2, space="PSUM") as psa_pool:
                for b in range(BPC):
                    diag = ascr.tile([64, 64], BF16, tag="diag")
                    nc.vector.tensor_scalar(
                        out=diag[:], in0=i64b[:], scalar1=rec[:, b:b + 1],
                        scalar2=None, op0=mybir.AluOpType.mult,
                    )
                    psat = psa_pool.tile([64, 64], F32, tag="psat")
                    nc.tensor.matmul(
                        psat[:], lhsT=expv[:, ts(b, S)], rhs=diag[:],
                        start=True, stop=True, skip_group_check=True,
                    )
                    nc.vector.tensor_copy(attn_sb[:, b:b + 1], psat[:, 63:64])
                    anT = ascr.tile([64, 64], BF16, tag="anT")
                    nc.vector.tensor_copy(anT[:], psat[:])
                    pswc = psa_pool.tile([128, KT * T], F32, tag="pswc")
                    for mt in range(KT):
                        nc.tensor.matmul(
                            pswc[:, ts(mt, T)], lhsT=ctxS[:, b, mt, :], rhs=anT[:],
                            start=True, stop=True, skip_group_check=True,
                        )
                    nc.vector.tensor_copy(
                        wcT[:, b, :, :], pswc[:].rearrange("p (k t) -> p k t", k=KT)
                    )
                nc.sync.dma_start(out=attn_d.ap(), in_=attn_sb[:])

            # ---- output projection + tanh ----
            with tc.tile_pool(name="psop", bufs=2, space="PSUM") as pso_pool:
                for mt in range(KT):
                    pso = pso_pool.tile([128, BPC * T], F32, tag="pso")
                    for kk in range(2 * KT):
                        if kk < KT:
                            rhs = wcT[:, :, kk, :]
                        else:
                            rhs = hist[:, :, kk - KT, :].rearrange("p t b -> p b t")
                        nc.tensor.matmul(
                            pso[:], lhsT=woutT[:, kk, mt, :], rhs=rhs,
                            start=(kk == 0), stop=(kk == 2 * KT - 1),
                            skip_group_check=True,
                        )
                    ot = ascr.tile([128, BPC * T], F32, tag="ot")
                    nc.scalar.activation(ot[:], pso[:],
                                         mybir.ActivationFunctionType.Tanh)
                    nc.sync.dma_start(out=out_d.ap()[ts(mt, 128), :], in_=ot[:])

    nc.compile()
    return nc


def _gate_rows(k):
    # [i_k | f_k | o_k | g_k] row indices in torch (i,f,g,o) stacked order
    return np.concatenate([
        np.arange(0 * H + k * 128, 0 * H + (k + 1) * 128),
        np.arange(1 * H + k * 128, 1 * H + (k + 1) * 128),
        np.arange(3 * H + k * 128, 3 * H + (k + 1) * 128),
        np.arange(2 * H + k * 128, 2 * H + (k + 1) * 128),
    ])


def _bf16(a):
    return np.asarray(a, dtype=np.float32).astype(ml_dtypes.bfloat16)


def kernel(input, h0, c0, context, emb, w_ih, w_hh, b_ih, b_hh, w_in, w_out,
           trace=False):
    input = np.asarray(input)
    h0 = np.asarray(h0, dtype=np.float32)
    c0 = np.asarray(c0, dtype=np.float32)
    context = np.asarray(context, dtype=np.float32)
    emb = np.asarray(emb, dtype=np.float32)
    w_ih = np.asarray(w_ih, dtype=np.float32)
    w_hh = np.asarray(w_hh, dtype=np.float32)
    b_ih = np.asarray(b_ih, dtype=np.float32)
    b_hh = np.asarray(b_hh, dtype=np.float32)
    w_in = np.asarray(w_in, dtype=np.float32)
    w_out = np.asarray(w_out, dtype=np.float32)

    if "nc" not in _cached:
        _cached["nc"] = _build_nc()
    nc = _cached["nc"]

    x = emb[input]                      # [T, B, E]
    xT = np.ascontiguousarray(np.transpose(x, (0, 2, 1)))  # [T, H, B]
    bias = b_ih + b_hh                  # [L, 4H]

    i128 = np.eye(128, dtype=np.float32)
    ones1 = np.ones((1, B), dtype=np.float32)
    hinitg = np.zeros((NC * 128, 128), dtype=np.float32)
    h0T = h0[0].T                       # [H, B]
    for k in range(NC):
        hinitg[k * 128:(k + 1) * 128, 0:64] = h0T[k * 128:(k + 1) * 128, :]

    winT_full = w_in.T                  # [H(k), H(m)]
    winT_tiles = winT_full.reshape(KT, 128, KT, 128).transpose(1, 0, 2, 3)
    woutT_full = w_out.T                # [2H(k), H(m)]
    woutT_tiles = woutT_full.reshape(2 * KT, 128, KT, 128).transpose(1, 0, 2, 3)

    in_maps = []
    for core in range(NC):
        rows = _gate_rows(core)
        b0v = bias[0][rows][None, :]
        b1v = bias[1][rows][None, :]
        wA = w_hh[0][rows].T            # [H, GS]
        wBm = np.concatenate([w_ih[1][rows].T, w_hh[1][rows].T], axis=0)
        wXm = w_ih[0][rows].T
        hsl = slice(core * 128, (core + 1) * 128)
        bsl = slice(core * BPC, (core + 1) * BPC)
        cinit = np.zeros((128, 128), dtype=np.float32)
        cinit[0:64] = c0[0][:, hsl]
        cinit[64:128] = c0[1][:, hsl]
        # ctxT: [p, kk, b, s] = context[s, b, kk*128+p]
        ctx_my = context[:, bsl, :]     # [S, BPC, H]
        ctxT = np.ascontiguousarray(
            ctx_my.transpose(2, 1, 0).reshape(KT, 128, BPC, S).transpose(1, 0, 2, 3)
        ).reshape(128, KT * BPC * S)
        # ctxS: [s, b, kk, m] = context[s, b, kk*128+m]
        ctxS = np.ascontiguousarray(
            ctx_my.transpose(0, 1, 2).reshape(S, BPC, KT, 128)
        ).reshape(S, BPC * KT * 128)

        in_maps.append({
            "wA": _bf16(wA), "wB": _bf16(wBm), "wX": _bf16(wXm),
            "b0": _bf16(b0v), "b1": _bf16(b1v),
            "ones1": _bf16(ones1), "i128": _bf16(i128),
            "xT": _bf16(xT.reshape(T * H, B)),
            "hinitg": _bf16(hinitg),
            "h1init": _bf16(h0[1][:, hsl]),
            "cinit": cinit,
            "ctxT": _bf16(ctxT), "ctxS": _bf16(ctxS),
            "winT": _bf16(winT_tiles.reshape(128, KT * KT * 128)),
            "woutT": _bf16(woutT_tiles.reshape(128, 2 * KT * KT * 128)),
        })

    global _last_in_maps
    _last_in_maps = in_maps
    res = run_bass_kernel_spmd(nc, in_maps, core_ids=list(range(NC)), trace=trace)
    results = res.results

    outs = np.zeros((T, B, H), dtype=np.float32)
    h_f = np.zeros((L, B, H), dtype=np.float32)
    c_f = np.zeros((L, B, H), dtype=np.float32)
    attn_last = np.zeros((B, S), dtype=np.float32)
    for core in range(NC):
        r = results[core]
        hsl = slice(core * 128, (core + 1) * 128)
        bsl = slice(core * BPC, (core + 1) * BPC)
        # out_o: [mt*128+p, b*T+t]
        o = r["out_o"].reshape(KT, 128, BPC, T)
        outs[:, bsl, :] = o.transpose(3, 2, 0, 1).reshape(T, BPC, H)
        hf = np.asarray(r["hf_o"], dtype=np.float32)
        h_f[0][:, hsl] = hf[0:64]
        h_f[1][:, hsl] = hf[64:128]
        cf = r["cf_o"]
        c_f[0][:, hsl] = cf[0:64]
        c_f[1][:, hsl] = cf[64:128]
        attn_last[bsl, :] = r["attn_o"].T
    if trace:
        return (outs, h_f, c_f, attn_last), res
    return outs, h_f, c_f, attn_last
